# revision 37
# baseline (speedup 1.0000x reference)
"""Trainium2 Bass kernel for a GCN critic network (2x GCNConv + host readout).

Strategy: graphs are 32 nodes with no cross-graph edges, so the GCN
scatter-add is a dense 32x32 normalized-adjacency matmul per graph. Four
graphs pack into one 128x128 block-diagonal stationary operand for the
TensorEngine. Data-parallel over 8 cores (512 graphs / 128 blocks per core).

v4: chunked wavefront, software-pipelined. The per-core input is one DRAM
tensor ("mega") with the small constants FIRST, then chunks of blocks
([A slab | A-host slab | x slab]); each chunk has its own DMA so compute
starts as soon as chunk 0 lands. Tick t of the pipeline runs P2 of chunk
t-1, P1 of chunk t, P3 of t-2 and P4+readout of t-3 so every engine has
independent work and cross-engine deps are at least a tick old. First and
last chunks are half-size to shorten pipeline fill/drain.

Per chunk (bf16 matmuls, fp32 PSUM):
  P1: a0t[f,d] per block      (lhsT=x_b [s,f], rhs=Ablk_b [s,d])
  P2: h1t = relu(W1^T @ a0t)  (feat-major, W1 halves stationary)
  P3: g_b = h1_b @ W2         (node-major, h1t halves stationary)
  P4: host rows only: lhsT = A-host [s, 52], two blocks packed per PSUM
      tile at partition bases 0/64 -> [116, 64] per block pair
  RO: mm = max(psum,0)*Wout-mask (relu fused into the PSUM read), reduce
      per 64-col group -> v[hostrow, pair]; final sel13 matmul sums the 13
      host rows per graph on the PE.
PSUM evictions are the bottleneck (GPSIMD has no PSUM port); they are
split across Scalar (h1 relu, first half of g) and Vector (a0t, second
half of g, STT, reduce).
"""

from contextlib import ExitStack

import numpy as np
import ml_dtypes

NG = 4096
NPG = 32
NH = 13
IN_DIM = 128
H1 = 256
H2 = 64
NCORES = 8
GPC = NG // NCORES          # graphs per core = 512
BPC = GPC // 4              # blocks per core = 128 (4 graphs / block)
NPC = GPC * NPG             # nodes per core = 16384

HC = 4 * NH                 # host cols per block = 52
# chunk sizes in blocks (sum = BPC); small edges for faster fill/drain
CHUNKS = [8] * 16
assert sum(CHUNKS) == BPC
NCHUNK = len(CHUNKS)
CBASE = [0] * NCHUNK        # first block of each chunk
for _i in range(1, NCHUNK):
    CBASE[_i] = CBASE[_i - 1] + CHUNKS[_i - 1]

# mega layout (elements per partition, bf16): consts first, then chunks
OFF_W1 = 0                  # [128, 256]
OFF_W2 = OFF_W1 + H1        # two [128,64] halves side by side
OFF_MASK = OFF_W2 + 2 * H2  # [104, 64] Wout rows by host index
OFF_SEL = OFF_MASK + H2     # [104, 8] host-group selector
CONST_W = 512
CHUNK_OFF = [0] * NCHUNK    # column offset of each chunk
_o = CONST_W
for _i in range(NCHUNK):
    CHUNK_OFF[_i] = _o
    _o += CHUNKS[_i] * (128 + HC + 128)
MEGA_W = _o

BF16 = ml_dtypes.bfloat16

_CACHE = {}


def _chunk_of(b):
    for c in range(NCHUNK):
        if b < CBASE[c] + CHUNKS[c]:
            return c
    raise ValueError(b)


def _a_off(b):
    c = _chunk_of(b)
    return CHUNK_OFF[c] + (b - CBASE[c]) * 128


def _ah_off(b):
    c = _chunk_of(b)
    return CHUNK_OFF[c] + CHUNKS[c] * 128 + (b - CBASE[c]) * HC


def _x_off(b):
    c = _chunk_of(b)
    return CHUNK_OFF[c] + CHUNKS[c] * (128 + HC) + (b - CBASE[c]) * 128


def _build_bass():
    import concourse.bass as bass
    import concourse.mybir as mybir
    import concourse.tile as tile
    from concourse import bacc
    from concourse.bass import ds

    bf = mybir.dt.bfloat16
    f32 = mybir.dt.float32
    AF = mybir.ActivationFunctionType
    ALU = mybir.AluOpType

    nc = bacc.Bacc("TRN2", target_bir_lowering=False, debug=False)

    megadev = nc.declare_dram_parameter("megadev", [128, MEGA_W], bf, isOutput=False)
    outdev = nc.declare_dram_parameter("outdev", [8, BPC // 2], f32, isOutput=True)

    with tile.TileContext(nc) as tc:
        ctx = ExitStack()
        ppool = ctx.enter_context(tc.tile_pool(name="persist", bufs=1))
        mega = ppool.tile([128, MEGA_W], bf, name="mega", tag="mega")
        v_all = ppool.tile([116, BPC // 2], f32, name="v_all", tag="v_all")
        vb = ppool.tile([116, BPC // 2], bf, name="vb", tag="vb")
        mask4 = ppool.tile([116, 4 * H2], bf, name="mask4", tag="mask4")
        out_sb = ppool.tile([8, BPC // 2], f32, name="out_sb", tag="out_sb")

        w1at = mega[:, ds(OFF_W1, 128)]
        w1bt = mega[:, ds(OFF_W1 + 128, 128)]
        w2at = mega[:, ds(OFF_W2, H2)]
        w2bt = mega[:, ds(OFF_W2 + H2, H2)]
        maskc = mega[0:116, ds(OFF_MASK, H2)]
        sel13 = mega[0:116, ds(OFF_SEL, 8)]

        # consts + chunk 0 ride in one DMA (adjacent in mega), then the
        # remaining chunks in consumption order
        n0 = CONST_W + CHUNKS[0] * (128 + HC + 128)
        nc.sync.dma_start(mega[:, ds(0, n0)], megadev[:, ds(0, n0)])
        for c in range(1, NCHUNK):
            off, n = CHUNK_OFF[c], CHUNKS[c] * (128 + HC + 128)
            nc.sync.dma_start(mega[:, ds(off, n)], megadev[:, ds(off, n)])

        psppool = ctx.enter_context(
            tc.tile_pool(name="psp", bufs=1, space="PSUM"))
        ps_rp = psppool.tile([128, 512], f32, name="ps_rp", tag="ps_rp")

        with tc.tile_pool(name="ps", bufs=1, space="PSUM") as pspool, \
             tc.tile_pool(name="ev", bufs=2) as evpool:

            # replicate Wout-mask 4x (DVE, waits only on the consts DMA)
            for j in range(4):
                nc.vector.tensor_scalar_add(mask4[:, ds(j * H2, H2)], maskc, 0.0)

            # zero the readout PSUM once so the zero-masked rows never read
            # stale NaN bits (halves alternate by chunk parity)
            nc.scalar.memzero(ps_rp[:, :])

            # Skewed pipeline (v6). Stage offsets per chunk c:
            #   P1(c)@c (a0t halves on DVE), P2g0(c)@c+2 (+h1p1 on ACT),
            #   P2g1(c)@c+2 (h1p2 deferred to c+3, first on ACT), P3(c)@c+4
            #   (g split DVE/ACT), P4(c)@c+6 (pR half by parity), h2(c)@c+7
            #   (DVE early, pR data is a tick old), mask-mul(c)@c+7 (GPSIMD),
            #   reduce(c)@c+8 (DVE first). pH is split into two 2-bank tiles
            #   so P2 group 0 only waits on h1p1's eviction, never on the
            #   whole h1 chain (that cycle paced v3-v5 at 3.6us/tick).
            FILL_T = 8

            def emit_p1(t):
                nb = CHUNKS[t]
                ps_a = pspool.tile([128, 1024], f32, tag="pA", bufs=1)
                a0t = evpool.tile([128, 1024], bf, tag="a0t", bufs=3)
                a0t_t[t] = a0t
                for j in range(nb):
                    b = CBASE[t] + j
                    nc.tensor.matmul(
                        ps_a[:, ds(128 * j, 128)],
                        lhsT=mega[:, ds(_x_off(b), 128)],
                        rhs=mega[:, ds(_a_off(b), 128)],
                        start=True, stop=True,
                    )
                nc.vector.tensor_scalar_add(a0t[:, ds(0, nb * 128)],
                                            ps_a[:, ds(0, nb * 128)], 0.0)

            a0t_t = {}
            h1_t = {}
            psh_t = {}
            g8_t = {}
            h2_t = {}
            f2_t = {}
            for t in range(NCHUNK + 7):
                # ---- reduce(t-6), ready since last tick (DVE first) ----
                if 6 <= t <= NCHUNK + 5 and (t - 6) < NCHUNK - 6:
                    c = t - 6
                    npair = CHUNKS[c] // 2
                    f2 = f2_t.pop(c)
                    nc.vector.tensor_reduce(
                        v_all[:, ds(CBASE[c] // 2, npair)],
                        f2[:, ds(0, npair * 16)].rearrange("p (k c) -> p k c", c=16),
                        axis=mybir.AxisListType.X,
                        op=ALU.add,
                    )

                # ---- h2(t-5): relu-evict pR half on DVE (data a tick old);
                # last two chunks read out directly on the idle-in-drain DVE
                if 5 <= t <= NCHUNK + 4:
                    c = t - 5
                    npair = CHUNKS[c] // 2
                    rbase = (c % 2) * 256
                    if c >= NCHUNK - 6:
                        mm2 = evpool.tile([116, 256], bf, tag="h2")
                        nc.vector.scalar_tensor_tensor(
                            mm2[:, ds(0, npair * H2)],
                            ps_rp[0:116, ds(rbase, npair * H2)],
                            0.0, mask4[:, ds(0, npair * H2)],
                            op0=ALU.max, op1=ALU.mult,
                        )
                        nc.vector.tensor_reduce(
                            v_all[:, ds(CBASE[c] // 2, npair)],
                            mm2[:, ds(0, npair * H2)].rearrange(
                                "p (k c) -> p k c", c=H2),
                            axis=mybir.AxisListType.X,
                            op=ALU.add,
                        )
                    else:
                        h2 = evpool.tile([116, 256], bf, tag="h2")
                        h2_t[c] = h2
                        nc.vector.tensor_scalar_max(
                            h2[:, ds(0, npair * H2)],
                            ps_rp[0:116, ds(rbase, npair * H2)], 0.0)

                # ---- h1p2(t-3) eviction, first on ACT ----
                if 3 <= t <= NCHUNK + 2 and CHUNKS[t - 3] > 4:
                    c = t - 3
                    ps_hb = psh_t.pop(c)
                    nc.scalar.activation(h1_t[c][:, ds(1024, 1024)],
                                         ps_hb[:, ds(0, 1024)], AF.Relu)

                # ---- P2(t-2) group 0 + h1p1 on ACT ----
                if 2 <= t <= NCHUNK + 1:
                    c = t - 2
                    nb = CHUNKS[c]
                    a0t = a0t_t[c]
                    ps_ha = pspool.tile([128, 1024], f32, tag="pHa", bufs=1)
                    h1 = evpool.tile([128, 2048], bf, tag="h1", bufs=3)
                    h1_t[c] = h1
                    w = min(512, nb * 128)
                    nc.tensor.matmul(ps_ha[:, ds(0, w)], lhsT=w1at,
                                     rhs=a0t[:, ds(0, w)], start=True, stop=True)
                    nc.tensor.matmul(ps_ha[:, ds(512, w)], lhsT=w1bt,
                                     rhs=a0t[:, ds(0, w)], start=True, stop=True)
                    nc.scalar.activation(h1[:, ds(0, 512 + w)], ps_ha[:, ds(0, 512 + w)],
                                         AF.Relu)

                # ---- P1(t): during DMA-paced fill ticks P1 is emitted
                # last so older chunks' stages aren't blocked in the PE
                # queue behind P1's wait for chunk t's DMA ----
                if t < NCHUNK and t > FILL_T:
                    emit_p1(t)

                # ---- P2(t-2) group 1 (h1p2 eviction deferred a tick) ----
                if 2 <= t <= NCHUNK + 1 and CHUNKS[t - 2] > 4:
                    c = t - 2
                    a0t = a0t_t.pop(c)
                    ps_hb = pspool.tile([128, 1024], f32, tag="pHb", bufs=1)
                    psh_t[c] = ps_hb
                    nc.tensor.matmul(ps_hb[:, ds(0, 512)], lhsT=w1at,
                                     rhs=a0t[:, ds(512, 512)], start=True, stop=True)
                    nc.tensor.matmul(ps_hb[:, ds(512, 512)], lhsT=w1bt,
                                     rhs=a0t[:, ds(512, 512)], start=True, stop=True)
                elif 2 <= t <= NCHUNK + 1:
                    a0t_t.pop(t - 2)

                # ---- GPSIMD mask-multiply + fold of t-5 (after DVE h2) ----
                if 5 <= t <= NCHUNK + 4 and (t - 5) < NCHUNK - 6:
                    c = t - 5
                    npair = CHUNKS[c] // 2
                    h2 = h2_t.pop(c)
                    mmw = evpool.tile([116, 256], bf, tag="mmw")
                    nc.gpsimd.tensor_mul(
                        mmw[:, ds(0, npair * H2)], h2[:, ds(0, npair * H2)],
                        mask4[:, ds(0, npair * H2)],
                    )
                    f1 = evpool.tile([116, 128], bf, tag="f1")
                    v_mm = mmw[:, ds(0, npair * H2)].rearrange(
                        "p (k c) -> p k c", c=H2)
                    nc.gpsimd.tensor_add(f1[:, ds(0, npair * 32)],
                                         v_mm[:, :, 0:32], v_mm[:, :, 32:64])
                    f2 = evpool.tile([116, 64], bf, tag="f2")
                    f2_t[c] = f2
                    v_f1 = f1[:, ds(0, npair * 32)].rearrange(
                        "p (k c) -> p k c", c=32)
                    nc.gpsimd.tensor_add(f2[:, ds(0, npair * 16)],
                                         v_f1[:, :, 0:16], v_f1[:, :, 16:32])

                # ---- P3(t-3): g per block ----
                if 3 <= t <= NCHUNK + 2:
                    c = t - 3
                    nb = CHUNKS[c]
                    h1 = h1_t.pop(c)
                    ps_g = pspool.tile([128, 512], f32, tag="pG", bufs=1)
                    for j in range(nb):
                        grp, o = j // 4, j % 4
                        la = h1[:, ds(grp * 1024 + o * 128, 128)]
                        lb = h1[:, ds(grp * 1024 + 512 + o * 128, 128)]
                        nc.tensor.matmul(ps_g[:, ds(j * H2, H2)], lhsT=la, rhs=w2at,
                                         start=True, stop=False)
                        nc.tensor.matmul(ps_g[:, ds(j * H2, H2)], lhsT=lb, rhs=w2bt,
                                         start=False, stop=True)
                    g8 = evpool.tile([128, 512], bf, tag="g8", bufs=3)
                    g8_t[c] = g8
                    nc.vector.tensor_scalar_add(g8[:, ds(0, nb * H2)],
                                                ps_g[:, ds(0, nb * H2)], 0.0)

                if t < NCHUNK and t <= FILL_T:
                    emit_p1(t)

                # ---- first output half once chunks 0..7 are reduced ----
                if t == NCHUNK:
                    nc.vector.tensor_scalar_add(vb[:, ds(0, 32)],
                                                v_all[:, ds(0, 32)], 0.0)
                    ops1 = pspool.tile([128, 1024], f32, tag="pA", bufs=1)
                    nc.tensor.matmul(ops1[0:8, ds(0, 32)], lhsT=sel13,
                                     rhs=vb[:, ds(0, 32)], start=True, stop=True)
                    nc.scalar.copy(out_sb[:, ds(0, 32)], ops1[0:8, ds(0, 32)])

                # ---- P4(t-4) host rows into the parity half of pR ----
                if 4 <= t <= NCHUNK + 3:
                    c = t - 4
                    nb = CHUNKS[c]
                    g8 = g8_t.pop(c)
                    rbase = (c % 2) * 256
                    for j in range(nb):
                        b = CBASE[c] + j
                        po = (j % 2) * 64
                        nc.tensor.matmul(
                            ps_rp[po:po + HC, ds(rbase + (j // 2) * H2, H2)],
                            lhsT=mega[:, ds(_ah_off(b), HC)],
                            rhs=g8[:, ds(j * H2, H2)],
                            start=True, stop=True)

            # ---- final: second output half ----
            nc.vector.tensor_scalar_add(vb[:, ds(32, 32)], v_all[:, ds(32, 32)], 0.0)
            nc.tensor.matmul(ps_rp[0:8, ds(0, 32)], lhsT=sel13,
                             rhs=vb[:, ds(32, 32)], start=True, stop=True)
            nc.scalar.copy(out_sb[:, ds(32, 32)], ps_rp[0:8, ds(0, 32)])
            nc.sync.dma_start(outdev[:, :], out_sb[:])

        ctx.close()

    nc.compile()
    return nc


def _prep_inputs(x, ei, host_idx, W1, b1, W2, b2, Wout, bout):
    """Host-side: dense per-graph adjacency, packed layouts, sharding.
    Returns (in_maps, bout_val) or None if structural assumptions fail."""
    x = np.asarray(x); ei = np.asarray(ei); host_idx = np.asarray(host_idx)
    W1 = np.asarray(W1); b1 = np.asarray(b1); W2 = np.asarray(W2)
    b2 = np.asarray(b2); Wout = np.asarray(Wout); bout = np.asarray(bout)

    N = NG * NPG
    src = ei[0].astype(np.int64)
    dst = ei[1].astype(np.int64)
    if (src // NPG != dst // NPG).any():
        return None
    hi = host_idx.reshape(NG, NH)
    if not (hi == (np.arange(NG)[:, None] * NPG + np.arange(NH)[None, :])).all():
        return None
    if b1.any() or b2.any():
        return None

    deg = np.bincount(dst, minlength=N).astype(np.float64) + 1.0
    dinv = 1.0 / np.sqrt(deg)
    A = np.zeros((NG, NPG, NPG), dtype=np.float64)
    g = src // NPG
    np.add.at(A, (g, dst % NPG, src % NPG), dinv[src] * dinv[dst])
    A[:, np.arange(NPG), np.arange(NPG)] += (dinv * dinv).reshape(NG, NPG)
    A32 = A.astype(np.float32)

    # host rows of a block pair pack at partition bases 0 (even block) and
    # 64 (odd block); rows 52..63 are dead and zero-masked
    WoutR = Wout[:, 0].reshape(NH, H2).astype(np.float32)
    mask2 = np.zeros((116, H2), dtype=np.float32)
    sel = np.zeros((116, 8), dtype=np.float32)
    for p in range(116):
        q = p if p < HC else p - 64
        if 0 <= q < HC:
            mask2[p] = WoutR[q % NH]
            sel[p, (0 if p < HC else 4) + q // NH] = 1.0

    w2re = np.empty((128, 2 * H2), dtype=np.float32)
    w2re[:, :H2] = W2[:128]
    w2re[:, H2:] = W2[128:]

    hostcols = (np.arange(4)[:, None] * NPG + np.arange(NH)[None, :]).ravel()

    in_maps = []
    for c in range(NCORES):
        mega = np.zeros((128, MEGA_W), dtype=np.float32)
        mega[:, OFF_W1:OFF_W1 + H1] = W1
        mega[:, OFF_W2:OFF_W2 + 2 * H2] = w2re
        mega[0:116, OFF_MASK:OFF_MASK + H2] = mask2
        mega[0:116, OFF_SEL:OFF_SEL + 8] = sel
        xc = x[c * NPC:(c + 1) * NPC].reshape(BPC, 128, IN_DIM)
        xc = np.ascontiguousarray(xc.transpose(1, 0, 2))       # [128, BPC, 128]
        Ac = A32[c * GPC:(c + 1) * GPC].reshape(BPC, 4, NPG, NPG)
        Ablk = np.zeros((BPC, 128, 128), dtype=np.float32)
        for j in range(4):
            # Ablk[b][s, d] = A[g][d_local, s_local]  (transposed within graph)
            Ablk[:, 32 * j:32 * (j + 1), 32 * j:32 * (j + 1)] = \
                Ac[:, j].transpose(0, 2, 1)
        Ablk = np.ascontiguousarray(Ablk.transpose(1, 0, 2))   # [128, BPC, 128]
        Ah = Ablk[:, :, hostcols]                              # [128, BPC, 52]
        for q in range(NCHUNK):
            nb = CHUNKS[q]
            bs = slice(CBASE[q], CBASE[q] + nb)
            base = CHUNK_OFF[q]
            mega[:, base:base + nb * 128] = Ablk[:, bs].reshape(128, nb * 128)
            mega[:, base + nb * 128:base + nb * (128 + HC)] = \
                Ah[:, bs].reshape(128, nb * HC)
            mega[:, base + nb * (128 + HC):base + nb * (256 + HC)] = \
                xc[:, bs].reshape(128, nb * 128)
        in_maps.append({"megadev": mega.astype(BF16)})
    return in_maps, float(bout[0])


def _numpy_fallback(x, ei, host_idx, W1, b1, W2, b2, Wout, bout):
    import jax
    jax.config.update("jax_platforms", "cpu")
    import jax.numpy as jnp

    def gcn_conv(xx, eei, W, b):
        Nn = xx.shape[0]
        loop = jnp.arange(Nn, dtype=eei.dtype)
        s = jnp.concatenate([eei[0], loop])
        d = jnp.concatenate([eei[1], loop])
        deg = jax.ops.segment_sum(jnp.ones(d.shape, dtype=xx.dtype), d, num_segments=Nn)
        dinv = jnp.where(deg > 0, jax.lax.rsqrt(deg), 0.0)
        norm = dinv[s] * dinv[d]
        h = xx @ W
        agg = jax.ops.segment_sum(h[s] * norm[:, None], d, num_segments=Nn)
        return agg + b

    h = jax.nn.relu(gcn_conv(jnp.asarray(x), jnp.asarray(ei), jnp.asarray(W1), jnp.asarray(b1)))
    h = jax.nn.relu(gcn_conv(h, jnp.asarray(ei), jnp.asarray(W2), jnp.asarray(b2)))
    host_z = h[jnp.asarray(host_idx)]
    nb = host_idx.shape[0] // NH
    z = host_z.reshape(nb, NH * h.shape[1])
    return np.asarray(z @ jnp.asarray(Wout) + jnp.asarray(bout))


def kernel(**inputs):
    prep = _prep_inputs(**inputs)
    if prep is None:
        return _numpy_fallback(**inputs)
    in_maps, bout_val = prep

    from concourse.bass_utils import run_bass_kernel_spmd

    if "nc" not in _CACHE:
        _CACHE["nc"] = _build_bass()
    nc = _CACHE["nc"]

    res = run_bass_kernel_spmd(nc, in_maps, core_ids=list(range(NCORES)))
    out = np.empty((NG, 1), dtype=np.float32)
    for c in range(NCORES):
        o = res.results[c]["outdev"]          # [8, BPC//2]; graph = pair*8 + j
        out[c * GPC:(c + 1) * GPC, 0] = o.T.ravel()
    out += bout_val
    return out
